# revision 50
# baseline (speedup 1.0000x reference)
"""Trainium2 Bass kernel for nn_MultiHeadAttention_66322884984909.

Math (faithful to reference):
  Q = X @ W_Q.T reshaped (B, H, L, hd) via DIRECT reshape -> head h owns rows
  128h:128(h+1) of the projected (L, D) matrix, reinterpreted as (L=2048,
  hd=64).  Heads are sequence-parallel: 32 (batch, head) pairs, 8 cores x 4.

Per pair (X_s = X[b, 128h:128h+128, :], shape (128, 1024)):
  Qf = X_s @ W_Q.T        (128, 1024)  -> Qh = Qf.reshape(2048, 64)
  S  = Qh @ Kh.T          (2048, 2048) causal-masked softmax (no scaling)
  O  = softmax(S) @ Vh    (2048, 64)
  Y  = O.reshape(128, 1024) @ W_O.T + b_O

All on-chip in bf16 (fp32 PSUM accumulation): projections produce the
"natural" (r, (t,w,d)) layout; PE transposes + strided engine copies convert
to Q^T/K^T/V^T (hd x L, l = 16r+t order) without any DRAM bounce.  V gets a
second transpose round into (l%128)-partition layout for PV.  Softmax has no
max-subtraction (|S| < ~60, exp fits fp32/bf16 range); row sums come free as
a 65th ones-column in V.  Diagonal causal masks are bf16 multiplies with two
precomputed 0/1 mask tiles.  Y runs with K=128 contraction using a
partition-shifted duplicate of O^T.
"""

import numpy as np
import ml_dtypes

import concourse.bass as bass
from concourse import bacc
import concourse.mybir as mybir
import concourse.tile as tile
from concourse.bass_utils import run_bass_kernel_spmd
from concourse.masks import make_identity

F32 = mybir.dt.float32
BF16 = mybir.dt.bfloat16
EXP = mybir.ActivationFunctionType.Exp
BF = ml_dtypes.bfloat16

B, L, D = 2, 2048, 1024
H, HD = 16, 64
NCORES = 8
PPC = 4  # pairs per core
NG = PPC // 2  # pair-groups per core


def build_nc(repeat=1, mode="full"):
    nc = bacc.Bacc(trn_type="TRN2", target_bir_lowering=False, debug=False)

    # xtg[g][p, kc, 128w + r] = X_pair(2g+w)[r, 128kc + p]
    xtg = nc.declare_dram_parameter("xtg", [NG, 128, 8, 256], BF16,
                                    isOutput=False)
    # w*[p, kc, j] = W.T[128kc + p, j]
    wq = nc.declare_dram_parameter("wq", [128, 8, 1024], BF16, isOutput=False)
    wk = nc.declare_dram_parameter("wk", [128, 8, 1024], BF16, isOutput=False)
    wv = nc.declare_dram_parameter("wv", [128, 8, 1024], BF16, isOutput=False)
    # wo[64t + d, 1024 k + col] = W_O.T[128k + 64t + d, col]
    wo = nc.declare_dram_parameter("wo", [128, 8192], BF16, isOutput=False)
    bias = nc.declare_dram_parameter("bias", [128, 1024], F32, isOutput=False)
    ones16 = nc.declare_dram_parameter("ones16", [128, 16], BF16,
                                       isOutput=False)
    out = nc.declare_dram_parameter("out", [PPC, 128, 1024], F32,
                                    isOutput=True)

    with tile.TileContext(nc) as tc:
      for _rep in range(repeat):
        with (
            tc.tile_pool(name="consts", bufs=1) as consts,
            tc.tile_pool(name="headt", bufs=1) as headt,
            tc.tile_pool(name="ps8", bufs=1, space="PSUM") as ps8,
            tc.tile_pool(name="natp", bufs=3) as natp,
            tc.tile_pool(name="onp", bufs=2) as onp,
            tc.tile_pool(name="ptp", bufs=4) as ptp,
            tc.tile_pool(name="rp", bufs=4) as rp,
            tc.tile_pool(name="yp", bufs=2) as ypool,
            tc.tile_pool(name="wp", bufs=2) as wpool,
        ):
            bias_sb = consts.tile([128, 1024], F32)
            nc.sync.dma_start(out=bias_sb, in_=bias[:])
            ident_f = consts.tile([128, 128], F32)
            make_identity(nc, ident_f)
            ident = consts.tile([128, 128], BF16)
            nc.vector.tensor_copy(ident, ident_f)
            ones_sb = consts.tile([128, 16], BF16)
            nc.sync.dma_start(out=ones_sb, in_=ones16[:])

            # single causal 0/1 triangle mask for the (128k x 128q) true
            # diagonal slice: keep where q - k >= 0
            ones_full = consts.tile([128, 128], BF16)
            nc.vector.memset(ones_full, 1.0)
            tri_mask = consts.tile([128, 128], BF16, tag="trimask",
                                   name="trimask")
            nc.gpsimd.affine_select(
                out=tri_mask, in_=ones_full,
                compare_op=mybir.AluOpType.is_ge,
                fill=0.0,
                base=0,
                pattern=[[1, 128]],
                channel_multiplier=-1,
            )
            zeros_pt = consts.tile([128, 512], BF16, tag="zeros_pt",
                                   name="zeros_pt")
            nc.vector.memset(zeros_pt, 0.0)
            ones64_f = consts.tile([1, 64], F32, tag="ones64f",
                                   name="ones64f")
            nc.vector.memset(ones64_f, 1.0)
            ones64 = consts.tile([1, 64], mybir.dt.float32r, tag="ones64",
                                 name="ones64")
            nc.vector.tensor_copy(ones64, ones64_f)

            xtg_sb = []
            for g in range(NG):
                t = consts.tile([128, 8, 256], BF16, tag=f"xtg{g}",
                                name=f"xtgsb{g}")
                nc.scalar.dma_start(out=t, in_=xtg[g])
                xtg_sb.append(t)

            # wo is consumed last (Y) -- load it on a separate queue, after
            # the xtg slabs, so it doesn't delay the first projection weights
            wo_sb = consts.tile([128, 8192], BF16, tag="wo", name="wo_sb")
            nc.gpsimd.dma_start(out=wo_sb, in_=wo[:])

            # head-transposed tensors, per group: partitions = (pair w, dim d)
            qht = [headt.tile([128, 2048], BF16, tag=f"qht{g}", name=f"qht{g}")
                   for g in range(NG)]
            kht = [headt.tile([128, 2048], BF16, tag=f"kht{g}", name=f"kht{g}")
                   for g in range(NG)]
            vt = [headt.tile([128, 2048], BF16, tag=f"vt{g}", name=f"vt{g}")
                  for g in range(NG)]
            # V in (l%128)-partition layout + ones column: vh[s, 65m+j]
            vh = [headt.tile([128, 16 * 65], BF16, tag=f"vh{p}", name=f"vh{p}")
                  for p in range(PPC)]
            # O^T per pair, duplicated partition-shifted for K=128 Y matmuls
            onorm = [onp.tile([128, 2048], BF16, tag=f"on{p}", name=f"on{p}")
                     for p in range(PPC)]

            def emit_phase(g, w_param, dstT, is_v):
                """Projection for both pairs of group g into dstT (=Q^T/K^T/
                V^T layout: dstT[64w + d, 16r + t] = proj_w[r, 64t + d]),
                then for V the second transpose round into vh."""
                w_sb = wpool.tile([128, 8, 1024], BF16, tag="w",
                                  name=f"w{g}_{id(w_param)}")
                # split across two DMA queues to halve the load latency the
                # first projection matmul waits on
                nc.sync.dma_start(out=w_sb[0:64, :, :],
                                  in_=w_param[0:64, :, :])
                nc.scalar.dma_start(out=w_sb[64:128, :, :],
                                    in_=w_param[64:128, :, :])
                nat = natp.tile([128, 16, 2, 64], BF16, tag="nat",
                                name=f"nat{g}_{is_v}")
                for w in range(2):
                    for jh in range(2):
                        ps = ps8.tile([128, 512], F32, tag="mm", bufs=2,
                                      name="projps")
                        for kc in range(8):
                            nc.tensor.matmul(
                                ps,
                                lhsT=xtg_sb[g][:, kc, 128 * w:128 * w + 128],
                                rhs=w_sb[:, kc, 512 * jh:512 * jh + 512],
                                start=(kc == 0), stop=(kc == 7),
                            )
                        # nat[r, t, w, d] for t in [8jh, 8jh+8)
                        nc.vector.tensor_copy(
                            nat[:, 8 * jh:8 * (jh + 1), w, :], ps)
                # T1: transpose each 128-col block t of nat -> ((w,d), r),
                # 4 blocks per PSUM tile, then one strided copy into dstT
                for bt in range(4):
                    tb = ps8.tile([128, 4, 128], BF16, tag="mm", bufs=2,
                                  name="t1ps")
                    for q in range(4):
                        t_ = 4 * bt + q
                        nc.tensor.transpose(
                            tb[:, q, :],
                            nat[:, t_, :, :],
                            ident,
                        )
                    # dstT[p, 16r + (4bt+q)] <- tb[p, q, r]
                    dst = dstT.rearrange("p (r t) -> p r t", t=16)
                    nc.scalar.copy(
                        dst[:, :, 4 * bt:4 * bt + 4],
                        tb.rearrange("p q r -> p r q"),
                    )
                if is_v:
                    # T2: per pair, transpose 128-l blocks of V^T into
                    # (l%128)-partition layout vh[s, 65m + d]
                    for w in range(2):
                        p = 2 * g + w
                        for half in range(2):
                            tb2 = ps8.tile([128, 8, 64], BF16, tag="mm",
                                           bufs=2, name="t2ps")
                            for ms in range(8):
                                m = 8 * half + ms
                                nc.tensor.transpose(
                                    tb2[:, ms, :],
                                    vt[g][64 * w:64 * w + 64,
                                          128 * m:128 * m + 128],
                                    ident[64 * w:64 * w + 64,
                                          64 * w:64 * w + 64],
                                )
                            vhr = vh[p].rearrange("p (m c) -> p m c", c=65)
                            nc.vector.tensor_copy(
                                vhr[:, 8 * half:8 * half + 8, 0:64], tb2)
                        # ones column
                        vhr = vh[p].rearrange("p (m c) -> p m c", c=65)
                        nc.vector.tensor_copy(vhr[:, :, 64], ones_sb)

            def emit_attention(g):
                # deferred normalization: (pv_sb, r1, i, a) emitted one
                # a-iteration later so the PE's broadcast matmul never waits
                # on the DVE recip chain
                pending = []

                def flush_pending():
                    for pv_sb, r1, i2, a2 in pending:
                        rb_ps = ps8.tile([64, 512], F32, tag="mm", bufs=2,
                                         name="rb_ps")
                        nc.tensor.matmul(rb_ps, lhsT=ones64, rhs=r1,
                                         start=True, stop=True)
                        nc.vector.tensor_mul(
                            onorm[2 * g + i2][0:64,
                                              a2 * 512:(a2 + 1) * 512],
                            pv_sb[0:64, :], rb_ps)
                    pending.clear()

                for a in range(4):
                    pvs = [ps8.tile([65, 512], F32, tag=f"pv{i}",
                                    name=f"pv_{i}") for i in range(2)]

                    def emit_pv(pt, i, bb, q2):
                        """PV for key block bb from pt columns q2*512..+512.
                        Diagonal blocks split into an unmasked bulk (columns
                        beyond the 128-wide true-diagonal slice) and the
                        masked slice, so the bulk never waits on the mask."""
                        vslice = vh[2 * g + i][:, bb * 65:bb * 65 + 65]
                        r0 = bb - 4 * a
                        c0 = q2 * 512
                        last = bb == 4 * a + 3
                        if r0 < 0:
                            nc.tensor.matmul(
                                pvs[i], lhsT=vslice, rhs=pt[:, c0:c0 + 512],
                                start=(bb == 0), stop=False,
                            )
                            return
                        # masked true-diagonal slice [128 r0, 128 r0 + 128)
                        sl = pt[:, c0 + 128 * r0:c0 + 128 * r0 + 128]
                        nc.vector.tensor_mul(sl, sl, tri_mask)
                        # unmasked bulk: columns >= 128 (r0 + 1)
                        if r0 < 3:
                            nc.tensor.matmul(
                                pvs[i][:, 128 * (r0 + 1):512],
                                lhsT=vslice,
                                rhs=pt[:, c0 + 128 * (r0 + 1):c0 + 512],
                                start=False, stop=False,
                                skip_group_check=True,
                            )
                        nc.tensor.matmul(
                            pvs[i][:, 128 * r0:128 * r0 + 128],
                            lhsT=vslice, rhs=sl,
                            start=False, stop=last,
                            skip_group_check=True,
                        )

                    if a == 0:
                        # open the accumulation group full-width with zeros
                        # (a=0 has only partial-width diagonal writes)
                        for i in range(2):
                            nc.tensor.matmul(
                                pvs[i], lhsT=vh[2 * g + i][:, 0:65],
                                rhs=zeros_pt, start=True, stop=False,
                            )

                    # one-gg software pipeline: PV consumes only the SBUF pt
                    # tile, so PV(gg-1) is emitted after S(gg)+exp(gg); the
                    # PE covers exp(gg) latency with PV(gg-1) work and the
                    # single-buffered st tiles never stall it
                    pending_pv = []
                    for gg in range(2 * a + 2):
                        if gg == 1:
                            flush_pending()
                        sts = [ps8.tile([128, 1024], F32, tag=t_,
                                        name=f"st{t_}")
                               for t_ in ("stA", "stB")]
                        for q2 in range(2):
                            bb = 2 * gg + q2
                            for i in range(2):
                                nc.tensor.matmul(
                                    sts[i][:, q2 * 512:(q2 + 1) * 512],
                                    lhsT=kht[g][64 * i:64 * i + 64,
                                                bb * 128:(bb + 1) * 128],
                                    rhs=qht[g][64 * i:64 * i + 64,
                                               a * 512:(a + 1) * 512],
                                    start=True, stop=True,
                                )
                        new_pv = []
                        for i in range(2):
                            pt = ptp.tile([128, 1024], BF16, tag="pt",
                                          bufs=4, name=f"pt_{i}")
                            nc.scalar.activation(pt, sts[i], EXP)
                            new_pv.append((pt, i, gg))
                        for pt, i, gg2 in pending_pv:
                            for q2 in range(2):
                                emit_pv(pt, i, 2 * gg2 + q2, q2)
                        pending_pv = new_pv
                    for pt, i, gg2 in pending_pv:
                        for q2 in range(2):
                            emit_pv(pt, i, 2 * gg2 + q2, q2)
                    for i in range(2):
                        # copy PV accumulator to SBUF (frees the PSUM slot
                        # for the next a-iteration), recip the ones-row
                        pv_sb = rp.tile([65, 512], F32, tag=f"pvsb{i}",
                                        bufs=2, name=f"pvsb{i}")
                        nc.vector.tensor_copy(pv_sb, pvs[i])
                        r1 = rp.tile([1, 512], mybir.dt.float32r,
                                     tag=f"r1_{i}", bufs=2, name=f"r1_{i}")
                        with nc.allow_low_precision(
                                reason="f32r recip keeps full f32 bits"):
                            nc.vector.reciprocal(r1, pv_sb[64:65, :])
                        pending.append((pv_sb, r1, i, a))
                flush_pending()
                for i in range(2):
                    p = 2 * g + i
                    # partition-shifted duplicate: onorm[64+d, l] = O^T[d, l+1]
                    nc.vector.tensor_copy(
                        onorm[p][64:128, 0:2047], onorm[p][0:64, 1:2048])

            def emit_y(g):
                for i in range(2):
                    p = 2 * g + i
                    on_r = onorm[p].rearrange("q (r t) -> q r t", t=16)
                    ysb = ypool.tile([128, 1024], F32, tag="ysb",
                                     name=f"ysb{p}")
                    for jh in range(2):
                        yps = ps8.tile([128, 512], F32, tag="mm", bufs=2,
                                       name=f"ypsum_{p}_{jh}")
                        for k8 in range(8):
                            nc.tensor.matmul(
                                yps,
                                lhsT=on_r[:, :, 2 * k8],
                                rhs=wo_sb[:, 1024 * k8 + 512 * jh:
                                          1024 * k8 + 512 * jh + 512],
                                start=(k8 == 0), stop=(k8 == 7),
                            )
                        nc.vector.tensor_add(
                            ysb[:, jh * 512:(jh + 1) * 512], yps,
                            bias_sb[:, jh * 512:(jh + 1) * 512])
                    nc.sync.dma_start(out=out[p], in_=ysb)

            for g in range(NG):
                emit_phase(g, wq, qht[g], False)
                emit_phase(g, wk, kht[g], False)
                emit_phase(g, wv, vt[g], True)
            if mode != "phases":
                # both Y stages after both attention groups: attn(g1) covers
                # g0's epilogue chain, and the Y matmuls run dependency-free
                for g in range(NG):
                    emit_attention(g)
                for g in range(NG):
                    emit_y(g)
            else:  # phases-only timing stub
                for p in range(PPC):
                    ysb = ypool.tile([128, 1024], F32, tag="ysb",
                                     name=f"ysbz{p}")
                    nc.vector.tensor_copy(ysb, bias_sb)
                    nc.sync.dma_start(out=out[p], in_=ysb)

    nc.finalize()
    return nc


def _host_prep(input_seq_embs, W_Q, W_K, W_V, W_O, b_O):
    X = np.asarray(input_seq_embs, dtype=np.float32)
    WQ = np.asarray(W_Q, dtype=np.float32)
    WK = np.asarray(W_K, dtype=np.float32)
    WV = np.asarray(W_V, dtype=np.float32)
    WO = np.asarray(W_O, dtype=np.float32)
    bO = np.asarray(b_O, dtype=np.float32)

    def warr(W):
        # (128, 8, 1024): w[p, kc, j] = W.T[128 kc + p, j]
        return np.ascontiguousarray(
            W.T.reshape(8, 128, 1024).transpose(1, 0, 2)).astype(BF)

    wq_arr = warr(WQ)
    wk_arr = warr(WK)
    wv_arr = warr(WV)
    wo_arr = np.ascontiguousarray(
        WO.T.reshape(8, 128, 1024).transpose(1, 0, 2).reshape(128, 8192)
    ).astype(BF)
    bias_arr = np.ascontiguousarray(
        np.broadcast_to(bO, (128, 1024)).astype(np.float32))
    ones_arr = np.ones((128, 16), dtype=BF)

    in_maps = []
    for c in range(NCORES):
        xtgs = []
        for g in range(NG):
            slabs = []
            for w in range(2):
                gp = PPC * c + 2 * g + w
                bb, hh = gp // H, gp % H
                # (128, 8, 128): [p, kc, r] = X[bb, 128 hh + r, 128 kc + p]
                slabT = X[bb, 128 * hh:128 * (hh + 1), :].T  # (1024, 128)
                slabs.append(slabT.reshape(8, 128, 128).transpose(1, 0, 2))
            xtgs.append(np.concatenate(slabs, axis=2))  # (128, 8, 256)
        in_maps.append({
            "xtg": np.ascontiguousarray(np.stack(xtgs)).astype(BF),
            "wq": wq_arr, "wk": wk_arr, "wv": wv_arr, "wo": wo_arr,
            "bias": bias_arr,
            "ones16": ones_arr,
        })
    return in_maps


_CACHED_NC = None


def get_nc():
    global _CACHED_NC
    if _CACHED_NC is None:
        _CACHED_NC = build_nc()
    return _CACHED_NC


def kernel(**inputs) -> np.ndarray:
    nc = get_nc()
    in_maps = _host_prep(**inputs)
    res = run_bass_kernel_spmd(nc, in_maps, list(range(NCORES)))
    out = np.empty((B, L, D), dtype=np.float32)
    for c in range(NCORES):
        y = res.results[c]["out"]  # (4, 128, 1024)
        for p in range(PPC):
            gp = PPC * c + p
            bb, hh = gp // H, gp % H
            out[bb, 128 * hh:128 * (hh + 1), :] = y[p]
    return out
